# revision 2
# baseline (speedup 1.0000x reference)
"""Trainium2 Bass kernel for an attention-augmented LSTM (CaptioningRNN).

Reference computation (per batch n, T timesteps):
    A_flat = A.reshape(N, H, 16); h0 = c0 = A_flat.mean(-1)
    scores_t = (h_{t-1} @ A_flat) / sqrt(H); w = softmax(scores)
    attn_t = A_flat @ w
    a = x_t @ Wx + h_{t-1} @ Wh + attn_t @ Wattn + b
    i, f, o, g = split(a, 4); c_t = sig(f)*c + sig(i)*tanh(g); h_t = sig(o)*tanh(c_t)

Strategy: data-parallel over batch across 8 cores (32 batch rows each).
Per core:
  Phase A: U = x @ Wx + b precomputed for all timesteps (bf16 weights,
           rows t-major) and staged to DRAM in bf16.
  Phase B: 64 recurrent steps. Gate matmul = [h; attn] (2048-dim contraction,
           bf16) against W2 = [Wh; Wattn] with gate-interleaved columns so each
           512-column block yields a full 128-dim slice of (i,f,o,g) and thus a
           128-dim slice of h/c. Attention scores are computed on the tensor
           engine (hT^T @ AT giving all batch pairs, diagonal extracted via a
           mask + strided reduce), softmax on ACT/DVE, attention pooling on DVE,
           h transposed back to hT layout via the DMA transpose xbar.

Weight-matrix column order (gate interleave): block j (512 cols) holds
original columns [i|f|o|g][j*128:(j+1)*128]. The same permutation is applied
to Wx, b and hence U.
"""

import math
import os

import numpy as np
import ml_dtypes

import concourse.bass as bass
import concourse.mybir as mybir
import concourse.tile as tile
from concourse import bacc

N, T, D, H = 256, 64, 1024, 1024
NCORES = 8
NB = N // NCORES          # 32 batch rows per core
G = 4 * H                 # 4096 gate columns
P = 16                    # attention positions (4x4)
KH = H // 128             # 8 contraction chunks for h
K2 = (2 * H) // 128       # 16 contraction chunks for [h; attn]
GB = G // 512             # 8 gate blocks of 512
F32 = mybir.dt.float32
BF16 = mybir.dt.bfloat16
BF = ml_dtypes.bfloat16

AF = mybir.ActivationFunctionType
ALU = mybir.AluOpType
AXX = mybir.AxisListType.X

_NC_CACHE = {}


def _gate_perm():
    """perm[new_col] = old_col for the gate-interleaved layout."""
    perm = np.empty(G, dtype=np.int64)
    for j in range(GB):
        for s in range(4):  # i, f, o, g
            perm[j * 512 + s * 128:(j * 512 + (s + 1) * 128)] = np.arange(
                s * H + j * 128, s * H + (j + 1) * 128)
    return perm


def build_nc(t_steps=T):
    """Build the SPMD Bass program (identical on all cores)."""
    nc = bacc.Bacc("TRN2", target_bir_lowering=False, debug=False,
                   num_devices=NCORES)

    xT_d = nc.dram_tensor("xT", [D, t_steps * NB], BF16, kind="ExternalInput")
    wx_d = nc.dram_tensor("wx", [D, G], BF16, kind="ExternalInput")
    w2_d = nc.dram_tensor("w2", [2 * H, G], BF16, kind="ExternalInput")
    b128_d = nc.dram_tensor("b128", [128, G], BF16, kind="ExternalInput")
    at_d = nc.dram_tensor("at", [H, NB * P], BF16, kind="ExternalInput")
    h0T_d = nc.dram_tensor("h0T", [H, NB], BF16, kind="ExternalInput")
    h0q_d = nc.dram_tensor("h0q", [2 * 128, 128], F32, kind="ExternalInput")
    mask_d = nc.dram_tensor("mask", [NB, NB * P], BF16, kind="ExternalInput")
    ident_d = nc.dram_tensor("ident", [128, 128], BF16, kind="ExternalInput")
    # quad-stacked bf16 output: row ((q*t + t)*128 + 32*gp + n), col c
    # holds h[n, t, (4q+gp)*128 + c]; host unshuffles + converts to f32
    out_d = nc.dram_tensor("out", [2 * t_steps * 128, 128], BF16,
                           kind="ExternalOutput")

    n_row_tiles = (t_steps * NB) // 128

    with tile.TileContext(nc) as tc:
        with tc.tile_pool(name="dram", bufs=1, space="DRAM") as dpool:
            # quad-stacked U: row (t*128 + gp*32 + n), col (q*512 + c)
            # holds U[t, n, gate block 4q+gp, c]
            u_dram = dpool.tile([t_steps * 128, 2 * 512], BF16)
            wdr = [dpool.tile([NB, P], BF16, tag=f"wdr{i}", name=f"wdr{i}")
                   for i in range(2)]

            # Phase A (U = x @ Wx + b) is streamed in block-units and
            # interleaved into the recurrence: x/Wx tiles are fetched from
            # DRAM on demand so both phases fit in SBUF together.
            with tc.tile_pool(name="res", bufs=1) as res, \
                 tc.tile_pool(name="ht", bufs=20) as htp, \
                 tc.tile_pool(name="u", bufs=2) as up, \
                 tc.tile_pool(name="st", bufs=2) as stp, \
                 tc.tile_pool(name="att", bufs=2) as attp, \
                 tc.tile_pool(name="abt", bufs=3) as abtp, \
                 tc.tile_pool(name="blk", bufs=2) as blkp, \
                 tc.tile_pool(name="pap", bufs=2) as pap, \
                 tc.tile_pool(name="pau", bufs=2) as pau, \
                 tc.tile_pool(name="psg", bufs=5, space="PSUM") as psg_p, \
                 tc.tile_pool(name="psa", bufs=2, space="PSUM") as pa_ps, \
                 tc.tile_pool(name="pss", bufs=1, space="PSUM") as pss_p:

                b128 = res.tile([128, G], BF16, tag="b128")
                nc.sync.dma_start(b128[:], b128_d[:])

                pa_state = {"m": -1, "xm": None}
                pa_fetched = {}

                def phase_a_fetch(m, g):
                    # stream x row-tile / Wx block from DRAM ahead of use
                    if pa_state["m"] != m:
                        xm = pap.tile([128, KH * 128], BF16, tag="xm",
                                      bufs=3)
                        for d_ in range(KH):
                            nc.gpsimd.dma_start(
                                xm[:, d_ * 128:(d_ + 1) * 128],
                                xT_d[d_ * 128:(d_ + 1) * 128,
                                     m * 128:(m + 1) * 128])
                        pa_state["m"], pa_state["xm"] = m, xm
                    wxg = pap.tile([128, KH * 512], BF16, tag="wxg",
                                   bufs=3)
                    for d_ in range(KH):
                        nc.gpsimd.dma_start(
                            wxg[:, d_ * 512:(d_ + 1) * 512],
                            wx_d[d_ * 128:(d_ + 1) * 128,
                                 g * 512:(g + 1) * 512])
                    pa_fetched[(m, g)] = (pa_state["xm"], wxg)

                def phase_a_compute(m, g):
                    # one (row-tile, gate-block) unit of U = x @ Wx + b
                    xm, wxg = pa_fetched.pop((m, g))
                    ps = pa_ps.tile([128, 512], F32, tag="ps")
                    for d_ in range(KH):
                        nc.tensor.matmul(ps[:], xm[:, d_ * 128:(d_ + 1) * 128],
                                         wxg[:, d_ * 512:(d_ + 1) * 512],
                                         start=(d_ == 0), stop=(d_ == KH - 1))
                    us = pau.tile([128, 512], BF16, tag="us")
                    nc.vector.tensor_add(us[:], ps[:], b128[:, g * 512:(g + 1) * 512])
                    q, gp = divmod(g, 4)
                    for r in range(4):
                        t_row = (4 * m + r) * 128 + gp * 32
                        nc.sync.dma_start(
                            u_dram[t_row:t_row + 32, q * 512:(q + 1) * 512],
                            us[r * 32:(r + 1) * 32, :])

                def phase_a_block(m, g):
                    phase_a_fetch(m, g)
                    phase_a_compute(m, g)

                # prologue phase A: tiles 0..3 (steps 0..15); the rest is
                # paced inside the step loop at 2 block-units per step
                PA_PRO = 4
                n_units = 8 * (n_row_tiles - PA_PRO)

                def pa_sched(t):
                    return range(min(2 * t, n_units),
                                 min(2 * (t + 1), n_units))

                for m in range(PA_PRO):
                    for g in range(GB):
                        phase_a_block(m, g)
                for j in pa_sched(0):
                    phase_a_fetch(PA_PRO + j // 8, j % 8)

                w2 = []
                for k in range(K2):
                    t_ = res.tile([128, G], BF16, tag=f"w2_{k}")
                    nc.sync.dma_start(t_[:], w2_d[k * 128:(k + 1) * 128, :])
                    w2.append(t_)
                at_all = res.tile([128, KH * NB * P], BF16, tag="at_all")
                for k in range(KH):
                    nc.sync.dma_start(
                        at_all[:, k * NB * P:(k + 1) * NB * P],
                        at_d[k * 128:(k + 1) * 128, :])
                at = [at_all[:, k * NB * P:(k + 1) * NB * P]
                      for k in range(KH)]
                mask = res.tile([NB, NB * P], BF16, tag="mask")
                nc.sync.dma_start(mask[:], mask_d[:])
                ident = res.tile([128, 128], BF16, tag="ident")
                nc.sync.dma_start(ident[:], ident_d[:])

                def ht_slices(tq):
                    return [tq[k // 4][:, 32 * (k % 4):32 * (k % 4) + 32]
                            for k in range(KH)]

                hTq = []
                for q in range(2):
                    t_ = htp.tile([128, 128], BF16, tag="htq", bufs=6)
                    for gp in range(4):
                        k = 4 * q + gp
                        nc.sync.dma_start(t_[:, 32 * gp:32 * gp + 32],
                                          h0T_d[k * 128:(k + 1) * 128, :])
                    hTq.append(t_)
                hT = ht_slices(hTq)
                c_b = []
                for q in range(2):
                    t_ = blkp.tile([128, 128], F32, tag="c", bufs=4,
                                   name=f"c0_{q}")
                    nc.sync.dma_start(t_[:], h0q_d[q * 128:(q + 1) * 128, :])
                    c_b.append(t_)

                u_t = up.tile([128, 2 * 512], BF16, tag="u")
                nc.sync.dma_start(u_t[:], u_dram[0:128, :])

                inv_sqrt_h = 1.0 / math.sqrt(H)

                def gsl(q, gp):
                    g = 4 * q + gp
                    return slice(g * 512, (g + 1) * 512)

                def smm(pq, gp, lhs, rhs, stop):
                    # accumulate into the 32-row strip of the quad bank
                    if isinstance(lhs, tile.Tile):
                        lhs = lhs[:]
                    nc.tensor.matmul(pq[32 * gp:32 * gp + 32, :], lhs, rhs,
                                     start=False, stop=stop,
                                     tile_position=(0, 32 * gp),
                                     skip_group_check=True)

                def umm(pq, u, q):
                    # seed the whole quad bank with U via identity matmul
                    nc.tensor.matmul(pq[:], ident[:],
                                     u[:, q * 512:(q + 1) * 512],
                                     start=True, stop=False,
                                     skip_group_check=True)

                # ---- prologue: scores S_0 + h-part of quad 0 ----
                ps_s = pss_p.tile([NB, NB * P], F32, tag="s")
                for k in range(KH):
                    nc.tensor.matmul(ps_s[:], hT[k], at[k],
                                     start=(k == 0), stop=(k == KH - 1))
                pq0 = psg_p.tile([128, 512], F32, tag="g", name="pq0")
                umm(pq0, u_t, 0)
                for k in range(KH):
                    for gp in range(4):
                        smm(pq0, gp, hT[k], w2[k][:, gsl(0, gp)], False)
                pq1 = psg_p.tile([128, 512], F32, tag="g", name="pq1")
                umm(pq1, u_t, 1)
                for k in range(KH):
                    for gp in range(4):
                        smm(pq1, gp, hT[k], w2[k][:, gsl(1, gp)], False)

                for t in range(t_steps):
                    last = (t + 1 >= t_steps)
                    if not last:
                        u_next = up.tile([128, 2 * 512], BF16, tag="u")
                        nc.scalar.dma_start(
                            u_next[:], u_dram[(t + 1) * 128:(t + 2) * 128, :])

                    # (a) softmax chain for step t (scores psum -> w1)
                    sm_sc = nc.enter_named_scope(f"sm{t}", False)
                    masked = stp.tile([NB, NB * P], F32, tag="masked", bufs=1)
                    nc.vector.tensor_tensor(
                        out=masked[:].rearrange("m (p n) -> m p n", n=NB),
                        in0=ps_s[:].rearrange("m (n p) -> m p n", p=P),
                        in1=mask[:].rearrange("m (n p) -> m p n", p=P),
                        op=ALU.mult)
                    sc = stp.tile([NB, P], F32, tag="sc")
                    nc.vector.tensor_reduce(
                        sc[:], masked[:].rearrange("m (p n) -> m p n", n=NB),
                        axis=AXX, op=ALU.add)
                    # exp(x) = s/(1-s) with s = sigmoid(x): keeps the ACT
                    # table cache at {Sigmoid, Tanh} with no per-step reloads
                    sg = stp.tile([NB, P], F32, tag="sg")
                    nc.scalar.activation(sg[:], sc[:], AF.Sigmoid,
                                         scale=float(inv_sqrt_h))
                    om = stp.tile([NB, P], F32, tag="om")
                    nc.scalar.activation(om[:], sc[:], AF.Sigmoid,
                                         scale=float(-inv_sqrt_h))
                    omr = stp.tile([NB, P], F32, tag="omr")
                    nc.vector.reciprocal(omr[:], om[:])
                    expw = stp.tile([NB, P], F32, tag="expw")
                    sume = stp.tile([NB, 1], F32, tag="sume")
                    nc.vector.scalar_tensor_tensor(
                        out=expw[:], in0=sg[:], scalar=1.0, in1=omr[:],
                        op0=ALU.mult, op1=ALU.mult, accum_out=sume[:])
                    rec = stp.tile([NB, 1], F32, tag="rec")
                    nc.vector.reciprocal(rec[:], sume[:])
                    w16 = stp.tile([NB, P], BF16, tag="w16")
                    nc.vector.tensor_scalar(out=w16[:], in0=expw[:],
                                            scalar1=rec[:], scalar2=None,
                                            op0=ALU.mult)
                    # flatten [NB, P] -> [1, NB*P]: direct SBUF->SBUF gather
                    w1 = stp.tile([1, NB * P], BF16, tag="w1")
                    nc.scalar.dma_start(w1[:], w16[:])
                    nc.leave_named_scope(f"sm{t}", sm_sc[0], False)

                    # (b) broadcast w to 128 partitions on gpsimd
                    sc_ = nc.enter_named_scope(f"att{t}", False)
                    wfull = attp.tile([128, NB * P], BF16, tag="wfull")
                    nc.gpsimd.partition_broadcast(wfull[:], w1[:])

                    # (e) attention pooling -> attnT: two wide DVE
                    # product+reduce pairs over the packed AT tile
                    attnT = []
                    with nc.allow_low_precision("attn pooled in bf16 anyway"):
                        for h in range(2):
                            hs = slice(h * 4 * NB * P, (h + 1) * 4 * NB * P)
                            pr = attp.tile([128, 4 * NB * P], BF16, tag="pr")
                            nc.vector.tensor_tensor(
                                out=pr[:].rearrange("m (k x) -> m k x", k=4),
                                in0=at_all[:, hs].rearrange(
                                    "m (k x) -> m k x", k=4),
                                in1=bass.AP(wfull[:].tensor, wfull[:].offset,
                                            [wfull[:].ap[0], [0, 4],
                                             wfull[:].ap[1]]),
                                op=ALU.mult)
                            ab4 = abtp.tile([128, 4 * NB], BF16, tag="ab")
                            nc.vector.tensor_reduce(
                                ab4[:],
                                pr[:].rearrange("m (k n p) -> m k n p", k=4,
                                                p=P),
                                axis=AXX, op=ALU.add)
                            for kk in range(4):
                                attnT.append(
                                    ab4[:, kk * NB:(kk + 1) * NB])
                    nc.leave_named_scope(f"att{t}", sc_[0], False)

                    # per-step state: blocks 4q..4q+3 of quad q live on
                    # partitions 32g'..32g'+31 of the quad's PSUM bank
                    c_new = [blkp.tile([128, 128], F32, tag="c", bufs=4,
                                       name=f"cn{q}_{t}") for q in range(2)]
                    hbf = [blkp.tile([128, 128], BF16, tag="hbf",
                                     name=f"hbf{q}_{t}") for q in range(2)]
                    hTq_new = []

                    def quad_math(q, pq):
                        # all four blocks of the quad, read from PSUM
                        sio = blkp.tile([128, 384], F32, tag="sio")
                        nc.scalar.activation(sio[:], pq[:, 0:384], AF.Sigmoid)
                        tg = blkp.tile([128, 128], F32, tag="tg")
                        nc.scalar.activation(tg[:], pq[:, 384:512], AF.Tanh)
                        m1 = blkp.tile([128, 128], F32, tag="m1")
                        nc.vector.tensor_tensor(out=m1[:], in0=sio[:, 0:128],
                                                in1=tg[:], op=ALU.mult)
                        m2 = blkp.tile([128, 128], F32, tag="m2")
                        nc.vector.tensor_tensor(out=m2[:], in0=sio[:, 128:256],
                                                in1=c_b[q][:], op=ALU.mult)
                        nc.vector.tensor_add(c_new[q][:], m1[:], m2[:])
                        tcn = blkp.tile([128, 128], F32, tag="tcn")
                        nc.scalar.activation(tcn[:], c_new[q][:], AF.Tanh)
                        nc.vector.tensor_tensor(out=hbf[q][:],
                                                in0=sio[:, 256:384],
                                                in1=tcn[:], op=ALU.mult)
                        if not last:
                            # one [128,128] transpose per quad: col block gp
                            # of the result is hT chunk 4q+gp
                            ht_ = htp.tile([128, 128], BF16, tag="htq",
                                           bufs=6)
                            nc.sync.dma_start(ht_[:], hbf[q][:],
                                              transpose=True)
                            hTq_new.append(ht_)
                        row = (q * t_steps + t) * 128
                        nc.sync.dma_start(out_d[row:row + 128, :], hbf[q][:])

                    # (f) attn-parts for quad 0 (k-outer, strips rotate),
                    # then quad0 math straight from PSUM
                    sc_ = nc.enter_named_scope(f"f05_{t}", False)
                    for k in range(KH, K2):
                        for gp in range(4):
                            smm(pq0, gp, attnT[k - KH], w2[k][:, gsl(0, gp)],
                                k == K2 - 1)
                    quad_math(0, pq0)
                    nc.leave_named_scope(f"f05_{t}", sc_[0], False)

                    # (h) attn-parts for quad 1 + quad1 math
                    sc_ = nc.enter_named_scope(f"h67_{t}", False)
                    for k in range(KH, K2):
                        for gp in range(4):
                            smm(pq1, gp, attnT[k - KH], w2[k][:, gsl(1, gp)],
                                k == K2 - 1)
                    quad_math(1, pq1)
                    nc.leave_named_scope(f"h67_{t}", sc_[0], False)

                    # (i..l) next step's scores + quad0 h-part;
                    # hT chunks 0..3 arrive with quad0, 4..7 with quad1
                    sc_ = nc.enter_named_scope(f"nxt{t}", False)
                    if not last:
                        hT_new = ht_slices(hTq_new)
                        ps_s = pss_p.tile([NB, NB * P], F32, tag="s")
                        for k in range(4):
                            nc.tensor.matmul(ps_s[:], hT_new[k], at[k],
                                             start=(k == 0), stop=False)
                        pq0n = psg_p.tile([128, 512], F32, tag="g",
                                          name=f"pq0_{t}")
                        umm(pq0n, u_next, 0)
                        for k in range(4):
                            for gp in range(4):
                                smm(pq0n, gp, hT_new[k],
                                    w2[k][:, gsl(0, gp)], False)
                        for k in (4, 5, 6, 7):
                            nc.tensor.matmul(ps_s[:], hT_new[k], at[k],
                                             start=False, stop=(k == 7))
                        for k in (4, 5, 6, 7):
                            for gp in range(4):
                                smm(pq0n, gp, hT_new[k],
                                    w2[k][:, gsl(0, gp)], False)
                        pq1n = psg_p.tile([128, 512], F32, tag="g",
                                          name=f"pq1_{t}")
                        umm(pq1n, u_next, 1)
                        for k in range(KH):
                            for gp in range(4):
                                smm(pq1n, gp, hT_new[k],
                                    w2[k][:, gsl(1, gp)], False)
                        pq0 = pq0n
                        pq1 = pq1n
                    nc.leave_named_scope(f"nxt{t}", sc_[0], False)

                    # paced phase-A filler: fetch one step ahead, compute
                    # this step's quota in the attn-wait window
                    for j in pa_sched(t + 1):
                        phase_a_fetch(PA_PRO + j // 8, j % 8)
                    for j in pa_sched(t):
                        phase_a_compute(PA_PRO + j // 8, j % 8)

                    if not last:
                        hT = hT_new
                        c_b = c_new
                        u_t = u_next

    nc.compile()
    return nc


def prepare_inputs(x, A, Wx, Wh, Wattn, b, t_steps=T):
    """Host-side sharding + layout prep. Returns list of per-core input maps."""
    x = np.asarray(x, dtype=np.float32)
    A = np.asarray(A, dtype=np.float32)
    Wx = np.asarray(Wx, dtype=np.float32)
    Wh = np.asarray(Wh, dtype=np.float32)
    Wattn = np.asarray(Wattn, dtype=np.float32)
    b = np.asarray(b, dtype=np.float32)

    perm = _gate_perm()
    wx_p = np.ascontiguousarray(Wx[:, perm]).astype(BF)
    w2_p = np.ascontiguousarray(np.vstack([Wh, Wattn])[:, perm]).astype(BF)
    b128 = np.ascontiguousarray(
        np.broadcast_to(b[perm], (128, G))).astype(BF)
    mask = np.zeros((NB, NB * P), dtype=BF)
    for n in range(NB):
        mask[n, n * P:(n + 1) * P] = 1
    ident = np.eye(128, dtype=BF)
    in_maps = []
    for c in range(NCORES):
        x_c = x[c * NB:(c + 1) * NB, :t_steps]          # (NB, t, D)
        xr = x_c.transpose(1, 0, 2).reshape(t_steps * NB, D)  # t-major rows
        xT = np.ascontiguousarray(xr.T).astype(BF)       # (D, t*NB)
        A_c = A[c * NB:(c + 1) * NB].reshape(NB, H, P)
        at_c = np.ascontiguousarray(
            A_c.transpose(1, 0, 2).reshape(H, NB * P)).astype(BF)
        h0 = A_c.mean(axis=2).astype(np.float32)         # (NB, H)
        h0T = np.ascontiguousarray(h0.T).astype(BF)      # (H, NB)
        # quad-stacked initial cell state: block g of quad q lives on
        # partitions 32*(g%4), columns = h dims within the block
        h0q = np.empty((2 * 128, 128), dtype=np.float32)
        for g in range(8):
            q, gp = divmod(g, 4)
            h0q[q * 128 + gp * 32:q * 128 + (gp + 1) * 32, :] = \
                h0[:, g * 128:(g + 1) * 128]
        in_maps.append({
            "xT": xT, "wx": wx_p, "w2": w2_p, "b128": b128,
            "at": at_c, "h0T": h0T, "h0q": h0q,
            "mask": mask, "ident": ident,
        })
    return in_maps


def kernel(x, A, Wx, Wh, Wattn, b):
    from concourse.bass_utils import run_bass_kernel_spmd

    key = T
    if key not in _NC_CACHE:
        _NC_CACHE[key] = build_nc(T)
    nc = _NC_CACHE[key]

    in_maps = prepare_inputs(x, A, Wx, Wh, Wattn, b)
    trace = bool(int(os.environ.get("KERNEL_TRACE", "0")))
    tmpdir = os.environ.get("KERNEL_TRACE_DIR") or None
    res = run_bass_kernel_spmd(nc, in_maps, core_ids=list(range(NCORES)),
                               trace=trace, tmpdir=tmpdir)
    kernel.last_result = res
    if res.exec_time_ns is not None:
        print(f"HW exec time: {res.exec_time_ns} ns")
        kernel.last_exec_time_ns = res.exec_time_ns
    # unshuffle quad-stacked bf16 output: buf[q, t, gp, n, c] ->
    # h[n, t, (4q+gp)*128 + c]
    outs = []
    for r in res.results:
        buf = np.asarray(r["out"]).reshape(2, T, 4, NB, 128)
        outs.append(buf.transpose(3, 1, 0, 2, 4).reshape(NB, T, H))
    return np.concatenate(outs, axis=0).astype(np.float32)


kernel.last_exec_time_ns = None



# revision 6
# speedup vs baseline: 1.2792x; 1.2792x over previous
"""Trainium2 Bass kernel for an attention-augmented LSTM (CaptioningRNN).

Reference computation (per batch n, T timesteps):
    A_flat = A.reshape(N, H, 16); h0 = c0 = A_flat.mean(-1)
    scores_t = (h_{t-1} @ A_flat) / sqrt(H); w = softmax(scores)
    attn_t = A_flat @ w
    a = x_t @ Wx + h_{t-1} @ Wh + attn_t @ Wattn + b
    i, f, o, g = split(a, 4); c_t = sig(f)*c + sig(i)*tanh(g); h_t = sig(o)*tanh(c_t)

Strategy: data-parallel over batch across 8 cores (32 batch rows each).
Per core:
  Phase A: U = x @ Wx + b, streamed in row-group-blocked units (Wx read 4x
           instead of 16x), staged to DRAM bf16, paced 2 units/step into the
           recurrence's idle windows.
  Phase B: 64 recurrent steps. Gate matmul = [h; attn] (2048-dim contraction,
           bf16) against W2 = [Wh; Wattn] with gate-interleaved columns so each
           512-column block yields a full 128-dim slice of (i,f,o,g) and thus a
           128-dim slice of h/c. Attention scores on the tensor engine (all
           batch pairs, diagonal via mask + strided reduce). Attention POOLING
           also on the tensor engine: softmax weights are scattered into a
           block-diagonal [128np, 4x32n] operand (stream-transpose + replicated
           DMA + static mask) and contracted against a (n,p)-major copy of A
           (atp), yielding attnT chunks directly in [h,n] layout. h transposed
           back to hT layout with PE transpose-mode matmuls (not DMA).

Weight-matrix column order (gate interleave): block j (512 cols) holds
original columns [i|f|o|g][j*128:(j+1)*128]. The same permutation is applied
to Wx, b and hence U.
"""

import math
import os

import numpy as np
import ml_dtypes

import concourse.bass as bass
import concourse.mybir as mybir
import concourse.tile as tile
from concourse import bacc

N, T, D, H = 256, 64, 1024, 1024
NCORES = 8
NB = N // NCORES          # 32 batch rows per core
G = 4 * H                 # 4096 gate columns
P = 16                    # attention positions (4x4)
KH = H // 128             # 8 contraction chunks for h
K2 = (2 * H) // 128       # 16 contraction chunks for [h; attn]
GB = G // 512             # 8 gate blocks of 512
NPC = (NB * P) // 128     # 4 chunks of the (n,p) axis
F32 = mybir.dt.float32
BF16 = mybir.dt.bfloat16
BF = ml_dtypes.bfloat16

AF = mybir.ActivationFunctionType
ALU = mybir.AluOpType
AXX = mybir.AxisListType.X

_NC_CACHE = {}

# phase A row-group blocking: 4 groups x 4 row-tiles
MG = 4                    # row-tile groups
MPG = 4                   # row-tiles per group (each row-tile = 4 timesteps)
N_ROW_TILES = MG * MPG    # 16


def _gate_perm():
    """perm[new_col] = old_col for the gate-interleaved layout."""
    perm = np.empty(G, dtype=np.int64)
    for j in range(GB):
        for s in range(4):  # i, f, o, g
            perm[j * 512 + s * 128:(j * 512 + (s + 1) * 128)] = np.arange(
                s * H + j * 128, s * H + (j + 1) * 128)
    return perm


def build_nc(t_steps=T):
    """Build the SPMD Bass program (identical on all cores)."""
    nc = bacc.Bacc("TRN2", target_bir_lowering=False, debug=False,
                   num_devices=NCORES)

    xT_d = nc.dram_tensor("xT", [D, t_steps * NB], BF16, kind="ExternalInput")
    wx_d = nc.dram_tensor("wx", [D, G], BF16, kind="ExternalInput")
    w2_d = nc.dram_tensor("w2", [2 * H, G], BF16, kind="ExternalInput")
    b128_d = nc.dram_tensor("b128", [128, G], BF16, kind="ExternalInput")
    at_d = nc.dram_tensor("at", [H, NB * P], BF16, kind="ExternalInput")
    atp_d = nc.dram_tensor("atp", [NB * P, H], BF16, kind="ExternalInput")
    h0T_d = nc.dram_tensor("h0T", [H, NB], BF16, kind="ExternalInput")
    h0q_d = nc.dram_tensor("h0q", [2 * 128, 128], F32, kind="ExternalInput")
    mask_d = nc.dram_tensor("mask", [NB, NB * P], BF16, kind="ExternalInput")
    m4_d = nc.dram_tensor("m4", [128, 128], BF16, kind="ExternalInput")
    ident_d = nc.dram_tensor("ident", [128, 128], BF16, kind="ExternalInput")
    # quad-stacked bf16 output: row ((q*t + t)*128 + 32*gp + n), col c
    # holds h[n, t, (4q+gp)*128 + c]; host unshuffles + converts to f32
    out_d = nc.dram_tensor("out", [2 * t_steps * 128, 128], BF16,
                           kind="ExternalOutput")

    with tile.TileContext(nc) as tc:
        with tc.tile_pool(name="dram", bufs=1, space="DRAM") as dpool:
            # quad-stacked U: row (t*128 + gp*32 + n), col (q*512 + c)
            # holds U[t, n, gate block 4q+gp, c]
            u_dram = dpool.tile([t_steps * 128, 2 * 512], BF16)
            # bounce buffer for the softmax-weight partition scatter
            wdrT = dpool.tile([16, 32], BF16)

            with tc.tile_pool(name="res", bufs=1) as res, \
                 tc.tile_pool(name="ht", bufs=20) as htp, \
                 tc.tile_pool(name="u", bufs=2) as up, \
                 tc.tile_pool(name="st", bufs=2) as stp, \
                 tc.tile_pool(name="att", bufs=2) as attp, \
                 tc.tile_pool(name="blk", bufs=2) as blkp, \
                 tc.tile_pool(name="pax", bufs=8) as paxp, \
                 tc.tile_pool(name="paw", bufs=2) as pawp, \
                 tc.tile_pool(name="pab", bufs=2) as pabp, \
                 tc.tile_pool(name="pau", bufs=2) as pau, \
                 tc.tile_pool(name="psg", bufs=4, space="PSUM") as psg_p, \
                 tc.tile_pool(name="psa", bufs=1, space="PSUM") as pa_ps, \
                 tc.tile_pool(name="psl", bufs=1, space="PSUM") as pl_ps, \
                 tc.tile_pool(name="pst", bufs=1, space="PSUM") as pt_ps, \
                 tc.tile_pool(name="pss", bufs=1, space="PSUM") as pss_p:

                # ---------------- phase A machinery ----------------
                # unit u = (mg, g, m): row-group mg, gate block g, local
                # row-tile m (global row-tile 4*mg + m). Order: mg, g, m.
                pa_xm = {}       # (mg, m) -> xm tile
                pa_wx = {}       # (mg, g) -> (wxg tile, b-slice tile)

                def pa_fetch_xm(mg, m):
                    if (mg, m) in pa_xm or mg >= MG:
                        return
                    xm = paxp.tile([128, KH * 128], BF16, tag="xm")
                    rt = 4 * mg + m
                    for d_ in range(KH):
                        nc.gpsimd.dma_start(
                            xm[:, d_ * 128:(d_ + 1) * 128],
                            xT_d[d_ * 128:(d_ + 1) * 128,
                                 rt * 128:(rt + 1) * 128])
                    pa_xm[(mg, m)] = xm

                def pa_fetch_unit(u):
                    mg, g = u // (GB * MPG), (u // MPG) % GB
                    if (mg, g) in pa_wx:
                        return
                    for m in range(MPG):       # xm for this group
                        pa_fetch_xm(mg, m)
                    if g >= 4:                 # trickle next group's xm
                        pa_fetch_xm(mg + 1, g - 4)
                    wxg = pawp.tile([128, KH * 512], BF16, tag="wxg")
                    for d_ in range(KH):
                        nc.gpsimd.dma_start(
                            wxg[:, d_ * 512:(d_ + 1) * 512],
                            wx_d[d_ * 128:(d_ + 1) * 128,
                                 g * 512:(g + 1) * 512])
                    bsl = pabp.tile([128, 512], BF16, tag="bsl")
                    nc.gpsimd.dma_start(bsl[:], b128_d[:, g * 512:(g + 1) * 512])
                    pa_wx[(mg, g)] = (wxg, bsl)

                def pa_compute_unit(u):
                    mg, g, m = u // (GB * MPG), (u // MPG) % GB, u % MPG
                    wxg, bsl = pa_wx[(mg, g)]
                    xm = pa_xm[(mg, m)]
                    ps = pa_ps.tile([128, 512], F32, tag="ps")
                    for d_ in range(KH):
                        nc.tensor.matmul(ps[:], xm[:, d_ * 128:(d_ + 1) * 128],
                                         wxg[:, d_ * 512:(d_ + 1) * 512],
                                         start=(d_ == 0), stop=(d_ == KH - 1))
                    us = pau.tile([128, 512], BF16, tag="us")
                    nc.vector.tensor_add(us[:], ps[:], bsl[:])
                    rt = 4 * mg + m
                    q, gp = divmod(g, 4)
                    for r in range(4):
                        t_row = (4 * rt + r) * 128 + gp * 32
                        nc.sync.dma_start(
                            u_dram[t_row:t_row + 32, q * 512:(q + 1) * 512],
                            us[r * 32:(r + 1) * 32, :])
                    # free consumed group resources at unit boundaries
                    if m == MPG - 1 and g == GB - 1:
                        for mm in range(MPG):
                            pa_xm.pop((mg, mm), None)
                    if m == MPG - 1:
                        pa_wx.pop((mg, g), None)

                N_UNITS = MG * GB * MPG          # 128
                PRO_UNITS = GB * MPG             # group 0 upfront

                def pa_sched(t):
                    return range(min(PRO_UNITS + 2 * t, N_UNITS),
                                 min(PRO_UNITS + 2 * (t + 1), N_UNITS))

                # ---------------- static tiles ----------------
                w2 = []
                for k in range(K2):
                    t_ = res.tile([128, G], BF16, tag=f"w2_{k}")
                    nc.sync.dma_start(t_[:], w2_d[k * 128:(k + 1) * 128, :])
                    w2.append(t_)
                at_all = res.tile([128, KH * NB * P], BF16, tag="at_all")
                for k in range(KH):
                    nc.sync.dma_start(
                        at_all[:, k * NB * P:(k + 1) * NB * P],
                        at_d[k * 128:(k + 1) * 128, :])
                at = [at_all[:, k * NB * P:(k + 1) * NB * P]
                      for k in range(KH)]
                atp = []
                for j in range(NPC):
                    t_ = res.tile([128, H], BF16, tag=f"atp{j}")
                    nc.sync.dma_start(t_[:], atp_d[j * 128:(j + 1) * 128, :])
                    atp.append(t_)
                mask = res.tile([NB, NB * P], BF16, tag="mask")
                nc.sync.dma_start(mask[:], mask_d[:])
                m4 = res.tile([128, 128], BF16, tag="m4")
                nc.sync.dma_start(m4[:], m4_d[:])
                ident = res.tile([128, 128], BF16, tag="ident")
                nc.sync.dma_start(ident[:], ident_d[:])
                w16rep = res.tile([128, 128], BF16, tag="w16rep")
                nc.vector.memset(w16rep[:], 0.0)

                # phase A prologue: group 0 complete (covers steps 0..15),
                # fetching one gate-block ahead of compute
                pa_fetch_unit(0)
                for u in range(PRO_UNITS):
                    pa_fetch_unit(u + MPG)
                    pa_compute_unit(u)
                for u in pa_sched(0):
                    pa_fetch_unit(u)

                def ht_slices(tq):
                    return [tq[k // 4][:, 32 * (k % 4):32 * (k % 4) + 32]
                            for k in range(KH)]

                hTq = []
                for q in range(2):
                    t_ = htp.tile([128, 128], BF16, tag="htq", bufs=6)
                    for gp in range(4):
                        k = 4 * q + gp
                        nc.sync.dma_start(t_[:, 32 * gp:32 * gp + 32],
                                          h0T_d[k * 128:(k + 1) * 128, :])
                    hTq.append(t_)
                hT = ht_slices(hTq)
                c_b = []
                for q in range(2):
                    t_ = blkp.tile([128, 128], F32, tag="c", bufs=4,
                                   name=f"c0_{q}")
                    nc.sync.dma_start(t_[:], h0q_d[q * 128:(q + 1) * 128, :])
                    c_b.append(t_)

                u_t = up.tile([128, 2 * 512], BF16, tag="u")
                nc.sync.dma_start(u_t[:], u_dram[0:128, :])

                inv_sqrt_h = 1.0 / math.sqrt(H)

                def gsl(q, gp):
                    g = 4 * q + gp
                    return slice(g * 512, (g + 1) * 512)

                def smm(pq, gp, lhs, rhs, stop):
                    # accumulate into the 32-row strip of the quad bank
                    if isinstance(lhs, tile.Tile):
                        lhs = lhs[:]
                    nc.tensor.matmul(pq[32 * gp:32 * gp + 32, :], lhs, rhs,
                                     start=False, stop=stop,
                                     tile_position=(0, 32 * gp),
                                     skip_group_check=True)

                def umm(pq, u, q):
                    # seed the whole quad bank with U via identity matmul
                    nc.tensor.matmul(pq[:], ident[:],
                                     u[:, q * 512:(q + 1) * 512],
                                     start=True, stop=False,
                                     skip_group_check=True)

                # ---- prologue: scores S_0 + h-parts of both quads ----
                ps_s = pss_p.tile([NB, NB * P], F32, tag="s")
                for k in range(KH):
                    nc.tensor.matmul(ps_s[:], hT[k], at[k],
                                     start=(k == 0), stop=(k == KH - 1))
                pq0 = psg_p.tile([128, 512], F32, tag="g", name="pq0")
                umm(pq0, u_t, 0)
                for k in range(KH):
                    for gp in range(4):
                        smm(pq0, gp, hT[k], w2[k][:, gsl(0, gp)], False)
                pq1 = psg_p.tile([128, 512], F32, tag="g", name="pq1")
                umm(pq1, u_t, 1)
                for k in range(KH):
                    for gp in range(4):
                        smm(pq1, gp, hT[k], w2[k][:, gsl(1, gp)], False)

                for t in range(t_steps):
                    last = (t + 1 >= t_steps)
                    if not last:
                        u_next = up.tile([128, 2 * 512], BF16, tag="u")
                        nc.scalar.dma_start(
                            u_next[:], u_dram[(t + 1) * 128:(t + 2) * 128, :])

                    # (a) softmax chain for step t (scores psum -> W16m)
                    sm_sc = nc.enter_named_scope(f"sm{t}", False)
                    masked = stp.tile([NB, NB * P], F32, tag="masked", bufs=1)
                    nc.vector.tensor_tensor(
                        out=masked[:].rearrange("m (p n) -> m p n", n=NB),
                        in0=ps_s[:].rearrange("m (n p) -> m p n", p=P),
                        in1=mask[:].rearrange("m (n p) -> m p n", p=P),
                        op=ALU.mult)
                    sc = stp.tile([NB, P], F32, tag="sc")
                    nc.vector.tensor_reduce(
                        sc[:], masked[:].rearrange("m (p n) -> m p n", n=NB),
                        axis=AXX, op=ALU.add)
                    # exp(x) = s/(1-s) with s = sigmoid(x): keeps the ACT
                    # table cache at {Sigmoid, Tanh} with no per-step reloads
                    sg = stp.tile([NB, P], F32, tag="sg")
                    nc.scalar.activation(sg[:], sc[:], AF.Sigmoid,
                                         scale=float(inv_sqrt_h))
                    om = stp.tile([NB, P], F32, tag="om")
                    nc.scalar.activation(om[:], sc[:], AF.Sigmoid,
                                         scale=float(-inv_sqrt_h))
                    omr = stp.tile([NB, P], F32, tag="omr")
                    nc.vector.reciprocal(omr[:], om[:])
                    expw = stp.tile([NB, P], F32, tag="expw")
                    sume = stp.tile([NB, 1], F32, tag="sume")
                    nc.vector.scalar_tensor_tensor(
                        out=expw[:], in0=sg[:], scalar=1.0, in1=omr[:],
                        op0=ALU.mult, op1=ALU.mult, accum_out=sume[:])
                    rec = stp.tile([NB, 1], F32, tag="rec")
                    nc.vector.reciprocal(rec[:], sume[:])
                    # normalized weights, bf16, into cols 0:16 of a padded
                    # [32,32] tile (cols 16:32 unused garbage)
                    w16p = stp.tile([NB, 32], BF16, tag="w16p")
                    nc.vector.tensor_scalar(out=w16p[:, 0:P], in0=expw[:],
                                            scalar1=rec[:], scalar2=None,
                                            op0=ALU.mult)
                    # wT[p, n] = w[n, p] (rows 16:32 garbage, never read)
                    wT = stp.tile([NB, 32], BF16, tag="wT")
                    nc.vector.transpose(wT[:], w16p[:])
                    # replicate wT[0:16, 8j:8j+8] down the partitions of the
                    # live 8-col window of block j via a DRAM bounce (the
                    # partition scatter is not affine-expressible in SBUF);
                    # static m4 mask then zeroes everything but the
                    # block-diagonal scatter w[n,p]
                    nc.scalar.dma_start(wdrT[:], wT[0:16, :])
                    wtd = wdrT[:]
                    rep_src = bass.AP(wtd.tensor, wtd.offset,
                                      [[0, 8], [32, 16], [8, NPC], [1, 8]])
                    wr = w16rep[:]
                    rep_dst = bass.AP(wr.tensor, wr.offset,
                                      [wr.ap[0], [40, NPC], [1, 8]])
                    nc.scalar.dma_start(rep_dst, rep_src)
                    w16m = attp.tile([128, 128], BF16, tag="w16m")
                    nc.vector.tensor_tensor(out=w16m[:], in0=w16rep[:],
                                            in1=m4[:], op=ALU.mult)
                    nc.leave_named_scope(f"sm{t}", sm_sc[0], False)

                    # (b) attention pooling on PE: attnT[k][c, n] =
                    #     sum_j atp_j[:, k]^T @ w16m[:, block j]
                    sc_ = nc.enter_named_scope(f"att{t}", False)
                    pool = pl_ps.tile([128, KH * 32], F32, tag="pool")
                    for k in range(KH):
                        for j in range(NPC):
                            nc.tensor.matmul(
                                pool[:, 32 * k:32 * k + 32],
                                atp[j][:, 128 * k:128 * (k + 1)],
                                w16m[:, 32 * j:32 * j + 32],
                                start=(j == 0), stop=(j == NPC - 1))
                    attn_sb = attp.tile([128, KH * 32], BF16, tag="attn_sb")
                    nc.vector.tensor_copy(attn_sb[:], pool[:])
                    attnT = [attn_sb[:, 32 * k:32 * k + 32]
                             for k in range(KH)]
                    nc.leave_named_scope(f"att{t}", sc_[0], False)

                    # per-step state: blocks 4q..4q+3 of quad q live on
                    # partitions 32g'..32g'+31 of the quad's PSUM bank
                    c_new = [blkp.tile([128, 128], F32, tag="c", bufs=4,
                                       name=f"cn{q}_{t}") for q in range(2)]
                    hbf = [blkp.tile([128, 128], BF16, tag="hbf",
                                     name=f"hbf{q}_{t}") for q in range(2)]

                    def quad_math(q, pq):
                        # all four blocks of the quad, read from PSUM
                        sio = blkp.tile([128, 384], F32, tag="sio")
                        nc.scalar.activation(sio[:], pq[:, 0:384], AF.Sigmoid)
                        tg = blkp.tile([128, 128], F32, tag="tg")
                        nc.scalar.activation(tg[:], pq[:, 384:512], AF.Tanh)
                        m1 = blkp.tile([128, 128], F32, tag="m1")
                        nc.vector.tensor_tensor(out=m1[:], in0=sio[:, 0:128],
                                                in1=tg[:], op=ALU.mult)
                        m2 = blkp.tile([128, 128], F32, tag="m2")
                        nc.vector.tensor_tensor(out=m2[:], in0=sio[:, 128:256],
                                                in1=c_b[q][:], op=ALU.mult)
                        nc.vector.tensor_add(c_new[q][:], m1[:], m2[:])
                        tcn = blkp.tile([128, 128], F32, tag="tcn")
                        nc.scalar.activation(tcn[:], c_new[q][:], AF.Tanh)
                        nc.vector.tensor_tensor(out=hbf[q][:],
                                                in0=sio[:, 256:384],
                                                in1=tcn[:], op=ALU.mult)
                        row = (q * t_steps + t) * 128
                        nc.sync.dma_start(out_d[row:row + 128, :], hbf[q][:])

                    # (f) attn-parts for quad 0 then quad0 math from PSUM
                    sc_ = nc.enter_named_scope(f"f05_{t}", False)
                    for k in range(KH, K2):
                        for gp in range(4):
                            smm(pq0, gp, attnT[k - KH], w2[k][:, gsl(0, gp)],
                                k == K2 - 1)
                    quad_math(0, pq0)
                    nc.leave_named_scope(f"f05_{t}", sc_[0], False)

                    # (h) attn-parts for quad 1 + quad1 math
                    sc_ = nc.enter_named_scope(f"h67_{t}", False)
                    for k in range(KH, K2):
                        for gp in range(4):
                            smm(pq1, gp, attnT[k - KH], w2[k][:, gsl(1, gp)],
                                k == K2 - 1)
                    quad_math(1, pq1)
                    nc.leave_named_scope(f"h67_{t}", sc_[0], False)

                    # phase-A filler inside the quad-math wait window
                    sc_ = nc.enter_named_scope(f"pa{t}", False)
                    for u in pa_sched(t + 1):
                        pa_fetch_unit(u)
                    for u in pa_sched(t):
                        pa_compute_unit(u)
                    nc.leave_named_scope(f"pa{t}", sc_[0], False)

                    # (i..l) PE transposes h->hT, next scores + h-parts
                    sc_ = nc.enter_named_scope(f"nxt{t}", False)
                    if not last:
                        hTq_new = []
                        for q in range(2):
                            tp = pt_ps.tile([128, 128], BF16, tag="tp")
                            nc.tensor.transpose(tp[:], hbf[q][:], ident[:])
                            ht_ = htp.tile([128, 128], BF16, tag="htq",
                                           bufs=6)
                            nc.vector.tensor_copy(ht_[:], tp[:])
                            hTq_new.append(ht_)
                        hT_new = ht_slices(hTq_new)
                        ps_s = pss_p.tile([NB, NB * P], F32, tag="s")
                        for k in range(KH):
                            nc.tensor.matmul(ps_s[:], hT_new[k], at[k],
                                             start=(k == 0), stop=(k == 7))
                        pq0n = psg_p.tile([128, 512], F32, tag="g",
                                          name=f"pq0_{t}")
                        umm(pq0n, u_next, 0)
                        for k in range(KH):
                            for gp in range(4):
                                smm(pq0n, gp, hT_new[k],
                                    w2[k][:, gsl(0, gp)], False)
                        pq1n = psg_p.tile([128, 512], F32, tag="g",
                                          name=f"pq1_{t}")
                        umm(pq1n, u_next, 1)
                        for k in range(KH):
                            for gp in range(4):
                                smm(pq1n, gp, hT_new[k],
                                    w2[k][:, gsl(1, gp)], False)
                        pq0 = pq0n
                        pq1 = pq1n
                    nc.leave_named_scope(f"nxt{t}", sc_[0], False)

                    if not last:
                        hT = hT_new
                        c_b = c_new
                        u_t = u_next

    nc.compile()
    return nc


def prepare_inputs(x, A, Wx, Wh, Wattn, b, t_steps=T):
    """Host-side sharding + layout prep. Returns list of per-core input maps."""
    x = np.asarray(x, dtype=np.float32)
    A = np.asarray(A, dtype=np.float32)
    Wx = np.asarray(Wx, dtype=np.float32)
    Wh = np.asarray(Wh, dtype=np.float32)
    Wattn = np.asarray(Wattn, dtype=np.float32)
    b = np.asarray(b, dtype=np.float32)

    perm = _gate_perm()
    wx_p = np.ascontiguousarray(Wx[:, perm]).astype(BF)
    w2_p = np.ascontiguousarray(np.vstack([Wh, Wattn])[:, perm]).astype(BF)
    b128 = np.ascontiguousarray(
        np.broadcast_to(b[perm], (128, G))).astype(BF)
    mask = np.zeros((NB, NB * P), dtype=BF)
    for n in range(NB):
        mask[n, n * P:(n + 1) * P] = 1
    # block-diagonal scatter mask: keeps (dn*16+p, 40j+dn) of the
    # replicated wT tile -> W16m[np', 32j + n] = w[n, p] for n = 8j+dn
    m4 = np.zeros((128, 128), dtype=BF)
    for j in range(NPC):
        for dn in range(8):
            for p in range(P):
                m4[dn * 16 + p, 40 * j + dn] = 1
    ident = np.eye(128, dtype=BF)
    in_maps = []
    for c in range(NCORES):
        x_c = x[c * NB:(c + 1) * NB, :t_steps]          # (NB, t, D)
        xr = x_c.transpose(1, 0, 2).reshape(t_steps * NB, D)  # t-major rows
        xT = np.ascontiguousarray(xr.T).astype(BF)       # (D, t*NB)
        A_c = A[c * NB:(c + 1) * NB].reshape(NB, H, P)
        at_c = np.ascontiguousarray(
            A_c.transpose(1, 0, 2).reshape(H, NB * P)).astype(BF)
        atp_c = np.ascontiguousarray(
            A_c.transpose(0, 2, 1).reshape(NB * P, H)).astype(BF)
        h0 = A_c.mean(axis=2).astype(np.float32)         # (NB, H)
        h0T = np.ascontiguousarray(h0.T).astype(BF)      # (H, NB)
        # quad-stacked initial cell state: block g of quad q lives on
        # partitions 32*(g%4), columns = h dims within the block
        h0q = np.empty((2 * 128, 128), dtype=np.float32)
        for g in range(8):
            q, gp = divmod(g, 4)
            h0q[q * 128 + gp * 32:q * 128 + (gp + 1) * 32, :] = \
                h0[:, g * 128:(g + 1) * 128]
        in_maps.append({
            "xT": xT, "wx": wx_p, "w2": w2_p, "b128": b128,
            "at": at_c, "atp": atp_c, "h0T": h0T, "h0q": h0q,
            "mask": mask, "m4": m4, "ident": ident,
        })
    return in_maps


def kernel(x, A, Wx, Wh, Wattn, b):
    from concourse.bass_utils import run_bass_kernel_spmd

    key = T
    if key not in _NC_CACHE:
        _NC_CACHE[key] = build_nc(T)
    nc = _NC_CACHE[key]

    in_maps = prepare_inputs(x, A, Wx, Wh, Wattn, b)
    trace = bool(int(os.environ.get("KERNEL_TRACE", "0")))
    tmpdir = os.environ.get("KERNEL_TRACE_DIR") or None
    res = run_bass_kernel_spmd(nc, in_maps, core_ids=list(range(NCORES)),
                               trace=trace, tmpdir=tmpdir)
    kernel.last_result = res
    if res.exec_time_ns is not None:
        print(f"HW exec time: {res.exec_time_ns} ns")
        kernel.last_exec_time_ns = res.exec_time_ns
    # unshuffle quad-stacked bf16 output: buf[q, t, gp, n, c] ->
    # h[n, t, (4q+gp)*128 + c]
    outs = []
    for r in res.results:
        buf = np.asarray(r["out"]).reshape(2, T, 4, NB, 128)
        outs.append(buf.transpose(3, 1, 0, 2, 4).reshape(NB, T, H))
    return np.concatenate(outs, axis=0).astype(np.float32)


kernel.last_exec_time_ns = None
